# revision 1
# baseline (speedup 1.0000x reference)
"""Multi-relation GAT layer on 8 Trainium2 NeuronCores.

Strategy: shard destination-node rows (i) across the 8 cores (256 rows each).
Host precomputes the cheap dense projections (Wh = H@W, attention vector dots
es/ed) in numpy, packs operands in device-friendly layouts, and the device
kernel does the heavy O(R*B*Hh*N^2) masked-softmax attention:

  scores are built directly in TRANSPOSED layout u^T[j (partition), i (free)]
  so the aggregation matmul contracts over j on the partition axis with no
  on-chip transpose of the big score tensor:

    s''[j,i] = (At_pre[j,i] + ed[j]) + es[i]     one fused scalar_tensor_tensor
               where At_pre = (A^T - 1)*BIG  (mask folded additively)
    l = Lrelu(s''), u = exp(l)                   ACT engine passes (bf16 out)
    agg^T[f,i] (+ rowsum row) = sum_j Wh^T u     PE accumulation matmuls, with
                                                 a ones column appended to Wh
  then small PE transposes bring agg back to [i, f] layout for the
  per-row softmax normalization, mean over relations, residual and LayerNorm.
"""

import sys

sys.path.insert(0, "/opt/trn_rl_repo")

import numpy as np
import ml_dtypes

R, B, N, D, Hh, hd = 3, 2, 2048, 128, 4, 32
RB = R * B
NCORES = 8
IS = N // NCORES  # 256 dst rows per core
NT = N // 128  # 16 j tiles
BIG = 200.0
LEAKY_ALPHA = 0.2
USE_LRELU = False  # exp(lrelu(x)) == max(exp(x), exp(alpha*x)) for alpha<1
LN_EPS = 1e-5
HW = Hh * 33  # 132 packed Wh cols per j-tile (32 wh + 1 ones per head)

_CACHE = {}


def _build_program():
    import concourse.bass as bass
    import concourse.mybir as mybir
    import concourse.tile as tile
    from concourse import bacc
    from concourse.masks import make_identity
    from contextlib import ExitStack

    f32 = mybir.dt.float32
    f16 = mybir.dt.float16
    bf16 = mybir.dt.bfloat16
    Alu = mybir.AluOpType
    Act = mybir.ActivationFunctionType

    nc = bacc.Bacc("TRN2", target_bir_lowering=False, debug=False)
    atp = nc.declare_dram_parameter("atp", [RB, 128, NT * IS], f16, isOutput=False)
    whp = nc.declare_dram_parameter("whp", [RB, 128, NT * HW], bf16, isOutput=False)
    es4 = nc.declare_dram_parameter("es4", [RB, 128, Hh * IS], f16, isOutput=False)
    edc = nc.declare_dram_parameter("edc", [RB, 128, Hh * NT], f32, isOutput=False)
    hres = nc.declare_dram_parameter("hres", [B * 2, 128, D], f32, isOutput=False)
    gmb = nc.declare_dram_parameter("gmb", [2, 128, D], f32, isOutput=False)
    out = nc.declare_dram_parameter("out", [B * 2, 128, D], f32, isOutput=True)

    with ExitStack() as ctx:
        tc = ctx.enter_context(tile.TileContext(nc))
        const = ctx.enter_context(tc.tile_pool(name="const", bufs=1))
        atp_pool = ctx.enter_context(tc.tile_pool(name="atp", bufs=2))
        ss_pool = ctx.enter_context(tc.tile_pool(name="ss", bufs=2))
        l_pool = ctx.enter_context(tc.tile_pool(name="lrelu", bufs=2))
        u_pool = ctx.enter_context(tc.tile_pool(name="u", bufs=3))
        aggsb_pool = ctx.enter_context(tc.tile_pool(name="aggsb", bufs=2))
        small = ctx.enter_context(tc.tile_pool(name="small", bufs=4))
        epi_pool = ctx.enter_context(tc.tile_pool(name="epi", bufs=2))
        psum_agg = ctx.enter_context(tc.tile_pool(name="pagg", bufs=1, space="PSUM"))
        psum_tp = ctx.enter_context(tc.tile_pool(name="ptp", bufs=2, space="PSUM"))

        # ---- constants / per-(r,b) operands ----
        ident = const.tile([128, 128], f32, tag="ident")
        make_identity(nc, ident[:])

        whp_sb, es4_sb, edc_sb = [], [], []
        for rb in range(RB):
            w = const.tile([128, NT * HW], bf16, tag=f"whp{rb}")
            nc.gpsimd.dma_start(w[:], whp[rb])
            whp_sb.append(w)
            e = const.tile([128, Hh * IS], f16, tag=f"es4{rb}")
            nc.gpsimd.dma_start(e[:], es4[rb])
            es4_sb.append(e)
            d = const.tile([128, Hh * NT], f32, tag=f"edc{rb}")
            nc.gpsimd.dma_start(d[:], edc[rb])
            edc_sb.append(d)

        hres_sb, acc = [], []
        for t in range(B * 2):
            hh = const.tile([128, D], f32, tag=f"hres{t}")
            nc.gpsimd.dma_start(hh[:], hres[t])
            hres_sb.append(hh)
            acc_t = const.tile([128, D], f32, tag=f"acc{t}", name=f"acc{t}")
            acc.append(acc_t)
        gam = const.tile([128, D], f32, tag="gam")
        nc.gpsimd.dma_start(gam[:], gmb[0])
        bet = const.tile([128, D], f32, tag="bet")
        nc.gpsimd.dma_start(bet[:], gmb[1])
        eps_b = const.tile([128, 1], f32, tag="eps_b")
        nc.gpsimd.memset(eps_b[:], LN_EPS)

        # ---- hot loop over (r, b) ----
        GJT = 4  # j-tiles per ss/u buffer
        for rb in range(RB):
            r, b = divmod(rb, B)
            a_sb = atp_pool.tile([128, NT * IS], f16, tag="atp")
            nc.gpsimd.dma_start(a_sb[:], atp[rb])

            aggp = [
                psum_agg.tile([33, IS], f32, tag=f"agg{h}", name=f"agg{h}")
                for h in range(Hh)
            ]

            for g in range(NT // GJT):
                ss = ss_pool.tile([128, GJT * Hh * IS], f16, tag="ss")
                for jl in range(GJT):
                    jt = g * GJT + jl
                    for h in range(Hh):
                        # s'' = (At_pre + ed[j]) + es[i]
                        nc.vector.scalar_tensor_tensor(
                            out=ss[:, (jl * Hh + h) * IS : (jl * Hh + h + 1) * IS],
                            in0=a_sb[:, jt * IS : (jt + 1) * IS],
                            scalar=edc_sb[rb][:, h * NT + jt : h * NT + jt + 1],
                            in1=es4_sb[rb][:, h * IS : (h + 1) * IS],
                            op0=Alu.add,
                            op1=Alu.add,
                        )
                u = u_pool.tile([128, GJT * Hh * IS], bf16, tag="u")
                if USE_LRELU:
                    lr = l_pool.tile([128, GJT * Hh * IS], f16, tag="lr")
                    nc.scalar.activation(lr[:], ss[:], Act.Lrelu, alpha=LEAKY_ALPHA)
                    nc.scalar.activation(u[:], lr[:], Act.Exp)
                else:
                    e1 = l_pool.tile([128, GJT * Hh * IS], bf16, tag="lr")
                    nc.scalar.activation(e1[:], ss[:], Act.Exp)
                    e2 = l_pool.tile([128, GJT * Hh * IS], bf16, tag="e2")
                    nc.scalar.activation(e2[:], ss[:], Act.Exp, scale=LEAKY_ALPHA)
                    nc.vector.tensor_max(u[:], e1[:], e2[:])
                for jl in range(GJT):
                    jt = g * GJT + jl
                    for h in range(Hh):
                        nc.tensor.matmul(
                            aggp[h][:, :],
                            lhsT=whp_sb[rb][:, jt * HW + h * 33 : jt * HW + (h + 1) * 33],
                            rhs=u[:, (jl * Hh + h) * IS : (jl * Hh + h + 1) * IS],
                            start=(jt == 0),
                            stop=(jt == NT - 1),
                        )

            # ---- per (rb, h): normalize by row-sums, accumulate over r ----
            for h in range(Hh):
                asb = aggsb_pool.tile([33, IS], f32, tag="aggsb")
                nc.scalar.copy(asb[:], aggp[h][:])
                for it in range(2):
                    tp = psum_tp.tile([128, 33], f32, tag="tp")
                    nc.tensor.transpose(
                        tp[:], asb[:, it * 128 : (it + 1) * 128], ident[:33, :33]
                    )
                    rec = small.tile([128, 1], f32, tag="rec")
                    nc.vector.reciprocal(rec[:], tp[:, 32:33])
                    contrib = small.tile([128, hd], f32, tag="contrib")
                    nc.vector.tensor_scalar(
                        out=contrib[:],
                        in0=tp[:, 0:32],
                        scalar1=rec[:],
                        scalar2=1.0 / R,
                        op0=Alu.mult,
                        op1=Alu.mult,
                    )
                    dst = acc[b * 2 + it][:, h * hd : (h + 1) * hd]
                    if r == 0:
                        nc.vector.tensor_copy(dst, contrib[:])
                    else:
                        nc.vector.tensor_add(dst, dst, contrib[:])

        # ---- epilogue: residual + LayerNorm ----
        for t in range(B * 2):
            x = epi_pool.tile([128, D], f32, tag="x")
            nc.vector.tensor_add(x[:], acc[t][:], hres_sb[t][:])
            mu = small.tile([128, 1], f32, tag="mu")
            nc.vector.reduce_sum(out=mu[:], in_=x[:], axis=mybir.AxisListType.X)
            nc.vector.tensor_scalar_mul(mu[:], mu[:], 1.0 / D)
            xc = epi_pool.tile([128, D], f32, tag="xc")
            nc.vector.tensor_scalar(
                out=xc[:], in0=x[:], scalar1=mu[:], scalar2=None, op0=Alu.subtract
            )
            sq = epi_pool.tile([128, D], f32, tag="sq")
            nc.vector.tensor_mul(sq[:], xc[:], xc[:])
            vs = small.tile([128, 1], f32, tag="vs")
            nc.vector.reduce_sum(out=vs[:], in_=sq[:], axis=mybir.AxisListType.X)
            nc.vector.tensor_scalar_mul(vs[:], vs[:], 1.0 / D)
            std = small.tile([128, 1], f32, tag="std")
            nc.scalar.activation(std[:], vs[:], Act.Sqrt, bias=eps_b[:])
            rstd = small.tile([128, 1], f32, tag="rstd")
            nc.vector.reciprocal(rstd[:], std[:])
            xn = epi_pool.tile([128, D], f32, tag="xn")
            nc.vector.tensor_scalar(
                out=xn[:], in0=xc[:], scalar1=rstd[:], scalar2=None, op0=Alu.mult
            )
            xg = epi_pool.tile([128, D], f32, tag="xg")
            nc.vector.tensor_mul(xg[:], xn[:], gam[:])
            xo = epi_pool.tile([128, D], f32, tag="xo")
            nc.vector.tensor_add(xo[:], xg[:], bet[:])
            nc.gpsimd.dma_start(out[t], xo[:])

    nc.compile()
    return nc


def _host_pack(H, A, W, a_src, a_dst, ln_gamma, ln_beta):
    H = np.asarray(H, np.float32)
    A = np.asarray(A)
    W = np.asarray(W, np.float32)
    a_src = np.asarray(a_src, np.float32)
    a_dst = np.asarray(a_dst, np.float32)
    ln_gamma = np.asarray(ln_gamma, np.float32)
    ln_beta = np.asarray(ln_beta, np.float32)

    Hm = H.reshape(B * N, D)
    # Wh[r,b,n,h,f]
    Wh = np.empty((R, B, N, Hh, hd), np.float32)
    for r in range(R):
        for h in range(Hh):
            Wh[r, :, :, h, :] = (Hm @ W[r, h]).reshape(B, N, hd)
    es = np.einsum("rbnhf,rhf->rbhn", Wh, a_src)  # [R,B,Hh,N]
    ed = np.einsum("rbnhf,rhf->rbhn", Wh, a_dst)

    # packed Wh + ones column, [RB, 128, NT*132] bf16
    whp = np.ones((RB, NT, 128, Hh, 33), np.float32)
    whp[:, :, :, :, :32] = Wh.reshape(RB, NT, 128, Hh, hd)
    whp = (
        whp.reshape(RB, NT, 128, HW)
        .transpose(0, 2, 1, 3)
        .reshape(RB, 128, NT * HW)
        .astype(ml_dtypes.bfloat16)
    )

    # ed columns [RB, 128, Hh*NT] f32: [p, h*NT+jt] = ed[r,b,h,jt*128+p]
    edc = (
        ed.reshape(RB, Hh, NT, 128).transpose(0, 3, 1, 2).reshape(RB, 128, Hh * NT)
    ).astype(np.float32)
    edc = np.ascontiguousarray(edc)

    # mask, transposed + pre-folded: (A^T - 1) * BIG, fp16  [R,B,j,i_all]
    At = A.transpose(0, 1, 3, 2)
    atp_full = ((At.astype(np.float32) - 1.0) * BIG).astype(np.float16)
    atp_full = atp_full.reshape(RB, NT, 128, N)

    gmbase = np.stack(
        [
            np.broadcast_to(ln_gamma, (128, D)),
            np.broadcast_to(ln_beta, (128, D)),
        ]
    ).astype(np.float32)
    gmbase = np.ascontiguousarray(gmbase)

    in_maps = []
    for c in range(NCORES):
        i0 = c * IS
        atp_c = np.ascontiguousarray(
            atp_full[:, :, :, i0 : i0 + IS].transpose(0, 2, 1, 3)
        ).reshape(RB, 128, NT * IS)
        es_c = es[:, :, :, i0 : i0 + IS].reshape(RB, Hh * IS).astype(np.float16)
        es_c = np.ascontiguousarray(
            np.broadcast_to(es_c[:, None, :], (RB, 128, Hh * IS))
        )
        hres_c = np.ascontiguousarray(H[:, i0 : i0 + IS, :]).reshape(B * 2, 128, D)
        in_maps.append(
            {
                "atp": atp_c,
                "whp": whp,
                "es4": es_c,
                "edc": edc,
                "hres": hres_c,
                "gmb": gmbase,
            }
        )
    return in_maps


def kernel(H, A, W, a_src, a_dst, ln_gamma, ln_beta):
    from concourse.bass_utils import run_bass_kernel_spmd

    if "nc" not in _CACHE:
        _CACHE["nc"] = _build_program()
    nc = _CACHE["nc"]

    in_maps = _host_pack(H, A, W, a_src, a_dst, ln_gamma, ln_beta)
    res = run_bass_kernel_spmd(nc, in_maps, list(range(NCORES)))

    full = np.empty((B, N, D), np.float32)
    for c in range(NCORES):
        o = np.asarray(res.results[c]["out"], np.float32).reshape(B, IS, D)
        full[:, c * IS : (c + 1) * IS, :] = o
    return full



# revision 4
# speedup vs baseline: 1.2252x; 1.2252x over previous
"""Multi-relation GAT layer on 8 Trainium2 NeuronCores.

Strategy: shard destination-node rows (i) across the 8 cores (256 rows each).
Host precomputes the dense projections (Wh = H@W, attention dots es/ed) and
the scalar exponentials; the device does the heavy O(R*B*Hh*N^2) masked
attention.

Key algebraic trick: scores are rank-1 before the leaky-relu
(s[i,j] = es_i + ed_j), and softmax over j is invariant to any per-i factor.
Dividing exp(lrelu(s)) by exp(es_i) gives

    u[j,i] = m[j,i] * max(q_j, t_j * g_i)
      q = exp(ed), t = exp(0.2*ed), g = exp(-0.8*es)   (all host-precomputed)

so the device never runs Exp over N^2 entries. Per j-tile/head:

    v = (g_b * t_j) max q_j     one tensor_scalar (4x DVE perf mode)
    u = v * m                   one tensor_tensor (2x DVE perf mode)
    agg^T[f,i] += Wh^T u        PE accumulation matmuls (ones column packed
                                into Wh gives the softmax denominators)

then small PE transposes bring agg back to [i, f] layout for the per-row
normalization, mean over relations, residual and LayerNorm.
"""

import sys

sys.path.insert(0, "/opt/trn_rl_repo")

import numpy as np

R, B, N, D, Hh, hd = 3, 2, 2048, 128, 4, 32
RB = R * B
NCORES = 8
IS = N // NCORES  # 256 dst rows per core
NT = N // 128  # 16 j tiles
LN_EPS = 1e-5
HW = Hh * 33  # 132 packed Wh cols per j-tile (32 wh + 1 ones per head)

_CACHE = {}


def _build_program():
    import concourse.bass as bass
    import concourse.mybir as mybir
    import concourse.tile as tile
    from concourse import bacc
    from concourse.masks import make_identity
    from contextlib import ExitStack

    f32 = mybir.dt.float32
    f16 = mybir.dt.float16
    Alu = mybir.AluOpType
    Act = mybir.ActivationFunctionType

    nc = bacc.Bacc("TRN2", target_bir_lowering=False, debug=False)
    mq = nc.declare_dram_parameter("mq", [RB, 128, NT * IS], f16, isOutput=False)
    whp = nc.declare_dram_parameter("whp", [RB, 128, NT * HW], f16, isOutput=False)
    gb4 = nc.declare_dram_parameter("gb4", [RB, 128, Hh * IS], f16, isOutput=False)
    qt = nc.declare_dram_parameter("qt", [RB, 128, 2 * Hh * NT], f32, isOutput=False)
    hres = nc.declare_dram_parameter("hres", [B * 2, 128, D], f32, isOutput=False)
    gmb = nc.declare_dram_parameter("gmb", [2, 128, D], f32, isOutput=False)
    out = nc.declare_dram_parameter("out", [B * 2, 128, D], f32, isOutput=True)

    with ExitStack() as ctx:
        tc = ctx.enter_context(tile.TileContext(nc))
        const = ctx.enter_context(tc.tile_pool(name="const", bufs=1))
        mq_pool = ctx.enter_context(tc.tile_pool(name="mq", bufs=2))
        v_pool = ctx.enter_context(tc.tile_pool(name="v", bufs=2))
        u_pool = ctx.enter_context(tc.tile_pool(name="u", bufs=3))
        aggsb_pool = ctx.enter_context(tc.tile_pool(name="aggsb", bufs=2))
        small = ctx.enter_context(tc.tile_pool(name="small", bufs=4))
        epi_pool = ctx.enter_context(tc.tile_pool(name="epi", bufs=2))
        psum_agg = ctx.enter_context(tc.tile_pool(name="pagg", bufs=2, space="PSUM"))
        psum_tp = ctx.enter_context(tc.tile_pool(name="ptp", bufs=2, space="PSUM"))

        # ---- constants / per-(r,b) operands ----
        ident = const.tile([128, 128], f32, tag="ident")
        make_identity(nc, ident[:])

        whp_sb, gb4_sb, qt_sb = [], [], []
        for rb in range(RB):
            w = const.tile([128, NT * HW], f16, tag=f"whp{rb}")
            nc.gpsimd.dma_start(w[:], whp[rb])
            whp_sb.append(w)
            g = const.tile([128, Hh * IS], f16, tag=f"gb4{rb}")
            nc.gpsimd.dma_start(g[:], gb4[rb])
            gb4_sb.append(g)
            q = const.tile([128, 2 * Hh * NT], f32, tag=f"qt{rb}")
            nc.gpsimd.dma_start(q[:], qt[rb])
            qt_sb.append(q)

        hres_sb, acc = [], []
        for t in range(B * 2):
            hh = const.tile([128, D], f32, tag=f"hres{t}")
            nc.gpsimd.dma_start(hh[:], hres[t])
            hres_sb.append(hh)
            acc_t = const.tile([128, D], f32, tag=f"acc{t}", name=f"acc{t}")
            acc.append(acc_t)
        gam = const.tile([128, D], f32, tag="gam")
        nc.gpsimd.dma_start(gam[:], gmb[0])
        bet = const.tile([128, D], f32, tag="bet")
        nc.gpsimd.dma_start(bet[:], gmb[1])
        eps_b = const.tile([128, 1], f32, tag="eps_b")
        nc.gpsimd.memset(eps_b[:], LN_EPS)

        # ---- hot loop over (r, b) ----
        GJT = 4  # j-tiles per v/u buffer
        for rb in range(RB):
            r, b = divmod(rb, B)
            m_sb = mq_pool.tile([128, NT * IS], f16, tag="mq")
            nc.gpsimd.dma_start(m_sb[:], mq[rb])

            aggp = psum_agg.tile([33, Hh * IS], f32, tag="agg", name=f"agg{rb}")

            for g in range(NT // GJT):
                v = v_pool.tile([128, GJT * Hh * IS], f16, tag="v")
                for jl in range(GJT):
                    jt = g * GJT + jl
                    for h in range(Hh):
                        # v = (g_i * t_j) max q_j
                        nc.vector.tensor_scalar(
                            out=v[:, (jl * Hh + h) * IS : (jl * Hh + h + 1) * IS],
                            in0=gb4_sb[rb][:, h * IS : (h + 1) * IS],
                            scalar1=qt_sb[rb][:, h * NT + jt : h * NT + jt + 1],
                            scalar2=qt_sb[rb][
                                :, Hh * NT + h * NT + jt : Hh * NT + h * NT + jt + 1
                            ],
                            op0=Alu.mult,
                            op1=Alu.max,
                        )
                u = u_pool.tile([128, GJT * Hh * IS], f16, tag="u")
                for jl in range(GJT):
                    jt = g * GJT + jl
                    for h in range(Hh):
                        nc.vector.tensor_mul(
                            u[:, (jl * Hh + h) * IS : (jl * Hh + h + 1) * IS],
                            v[:, (jl * Hh + h) * IS : (jl * Hh + h + 1) * IS],
                            m_sb[:, jt * IS : (jt + 1) * IS],
                        )
                for jl in range(GJT):
                    jt = g * GJT + jl
                    for h in range(Hh):
                        nc.tensor.matmul(
                            aggp[:, h * IS : (h + 1) * IS],
                            lhsT=whp_sb[rb][:, jt * HW + h * 33 : jt * HW + (h + 1) * 33],
                            rhs=u[:, (jl * Hh + h) * IS : (jl * Hh + h + 1) * IS],
                            start=(jt == 0),
                            stop=(jt == NT - 1),
                        )

            # ---- per (rb, h): normalize by row-sums, accumulate over r ----
            asb = aggsb_pool.tile([33, Hh * IS], f32, tag="aggsb")
            nc.scalar.copy(asb[:], aggp[:])
            for h in range(Hh):
                for it in range(2):
                    tp = psum_tp.tile([128, 33], f32, tag="tp")
                    nc.tensor.transpose(
                        tp[:],
                        asb[:, h * IS + it * 128 : h * IS + (it + 1) * 128],
                        ident[:33, :33],
                    )
                    rec = small.tile([128, 1], f32, tag="rec")
                    nc.vector.reciprocal(rec[:], tp[:, 32:33])
                    contrib = small.tile([128, hd], f32, tag="contrib")
                    nc.vector.tensor_scalar(
                        out=contrib[:],
                        in0=tp[:, 0:32],
                        scalar1=rec[:],
                        scalar2=1.0 / R,
                        op0=Alu.mult,
                        op1=Alu.mult,
                    )
                    dst = acc[b * 2 + it][:, h * hd : (h + 1) * hd]
                    if r == 0:
                        nc.vector.tensor_copy(dst, contrib[:])
                    else:
                        nc.vector.tensor_add(dst, dst, contrib[:])

        # ---- epilogue: residual + LayerNorm ----
        for t in range(B * 2):
            x = epi_pool.tile([128, D], f32, tag="x")
            nc.vector.tensor_add(x[:], acc[t][:], hres_sb[t][:])
            mu = small.tile([128, 1], f32, tag="mu")
            nc.vector.reduce_sum(out=mu[:], in_=x[:], axis=mybir.AxisListType.X)
            nc.vector.tensor_scalar_mul(mu[:], mu[:], 1.0 / D)
            xc = epi_pool.tile([128, D], f32, tag="xc")
            nc.vector.tensor_scalar(
                out=xc[:], in0=x[:], scalar1=mu[:], scalar2=None, op0=Alu.subtract
            )
            sq = epi_pool.tile([128, D], f32, tag="sq")
            nc.vector.tensor_mul(sq[:], xc[:], xc[:])
            vs = small.tile([128, 1], f32, tag="vs")
            nc.vector.reduce_sum(out=vs[:], in_=sq[:], axis=mybir.AxisListType.X)
            nc.vector.tensor_scalar_mul(vs[:], vs[:], 1.0 / D)
            std = small.tile([128, 1], f32, tag="std")
            nc.scalar.activation(std[:], vs[:], Act.Sqrt, bias=eps_b[:])
            rstd = small.tile([128, 1], f32, tag="rstd")
            nc.vector.reciprocal(rstd[:], std[:])
            xn = epi_pool.tile([128, D], f32, tag="xn")
            nc.vector.tensor_scalar(
                out=xn[:], in0=xc[:], scalar1=rstd[:], scalar2=None, op0=Alu.mult
            )
            xg = epi_pool.tile([128, D], f32, tag="xg")
            nc.vector.tensor_mul(xg[:], xn[:], gam[:])
            xo = epi_pool.tile([128, D], f32, tag="xo")
            nc.vector.tensor_add(xo[:], xg[:], bet[:])
            nc.gpsimd.dma_start(out[t], xo[:])

    nc.compile()
    return nc


def _host_pack(H, A, W, a_src, a_dst, ln_gamma, ln_beta):
    H = np.asarray(H, np.float32)
    A = np.asarray(A)
    W = np.asarray(W, np.float32)
    a_src = np.asarray(a_src, np.float32)
    a_dst = np.asarray(a_dst, np.float32)
    ln_gamma = np.asarray(ln_gamma, np.float32)
    ln_beta = np.asarray(ln_beta, np.float32)

    Hm = H.reshape(B * N, D)
    # Wh[r,b,n,h,f]
    Wh = np.empty((R, B, N, Hh, hd), np.float32)
    for r in range(R):
        for h in range(Hh):
            Wh[r, :, :, h, :] = (Hm @ W[r, h]).reshape(B, N, hd)
    es = np.einsum("rbnhf,rhf->rbhn", Wh, a_src)  # [R,B,Hh,N]
    ed = np.einsum("rbnhf,rhf->rbhn", Wh, a_dst)

    # packed Wh + ones column, [RB, 128, NT*132] f16
    whp = np.ones((RB, NT, 128, Hh, 33), np.float32)
    whp[:, :, :, :, :32] = Wh.reshape(RB, NT, 128, Hh, hd)
    whp = (
        whp.reshape(RB, NT, 128, HW)
        .transpose(0, 2, 1, 3)
        .reshape(RB, 128, NT * HW)
        .astype(np.float16)
    )

    # scalar factor columns [RB, 128, 2*Hh*NT] f32:
    #   [p, h*NT+jt]         = t = exp(0.2*ed[r,b,h,jt*128+p])
    #   [p, Hh*NT + h*NT+jt] = q = exp(ed[r,b,h,jt*128+p])
    edc = ed.reshape(RB, Hh, NT, 128).transpose(0, 3, 1, 2)  # [RB,128,Hh,NT]
    qt = np.empty((RB, 128, 2 * Hh * NT), np.float32)
    qt[:, :, : Hh * NT] = np.exp(0.2 * edc).reshape(RB, 128, Hh * NT)
    qt[:, :, Hh * NT :] = np.exp(edc).reshape(RB, 128, Hh * NT)
    qt = np.ascontiguousarray(qt)

    # raw 0/1 mask, transposed: [R,B,j,i_all] fp16
    At = A.transpose(0, 1, 3, 2)
    mq_full = At.astype(np.float16).reshape(RB, NT, 128, N)

    # g = exp(-0.8*es), broadcast across partitions
    g_all = np.exp(-0.8 * es).astype(np.float16)  # [R,B,Hh,N]

    gmbase = np.stack(
        [
            np.broadcast_to(ln_gamma, (128, D)),
            np.broadcast_to(ln_beta, (128, D)),
        ]
    ).astype(np.float32)
    gmbase = np.ascontiguousarray(gmbase)

    in_maps = []
    for c in range(NCORES):
        i0 = c * IS
        mq_c = np.ascontiguousarray(
            mq_full[:, :, :, i0 : i0 + IS].transpose(0, 2, 1, 3)
        ).reshape(RB, 128, NT * IS)
        g_c = g_all[:, :, :, i0 : i0 + IS].reshape(RB, Hh * IS)
        g_c = np.ascontiguousarray(np.broadcast_to(g_c[:, None, :], (RB, 128, Hh * IS)))
        hres_c = np.ascontiguousarray(H[:, i0 : i0 + IS, :]).reshape(B * 2, 128, D)
        in_maps.append(
            {
                "mq": mq_c,
                "whp": whp,
                "gb4": g_c,
                "qt": qt,
                "hres": hres_c,
                "gmb": gmbase,
            }
        )
    return in_maps


def kernel(H, A, W, a_src, a_dst, ln_gamma, ln_beta):
    from concourse.bass_utils import run_bass_kernel_spmd

    if "nc" not in _CACHE:
        _CACHE["nc"] = _build_program()
    nc = _CACHE["nc"]

    in_maps = _host_pack(H, A, W, a_src, a_dst, ln_gamma, ln_beta)
    res = run_bass_kernel_spmd(nc, in_maps, list(range(NCORES)))

    full = np.empty((B, N, D), np.float32)
    for c in range(NCORES):
        o = np.asarray(res.results[c]["out"], np.float32).reshape(B, IS, D)
        full[:, c * IS : (c + 1) * IS, :] = o
    return full
